# revision 12
# baseline (speedup 1.0000x reference)
"""DenseGCNBlock on 8 trn2 NeuronCores (Bass/Tile) — dense-adjacency version.

Math: reference computes, per layer l with weight W_l [C_l+16, 128]:
    msg_e = concat(cat[src_e], ea_e) @ W_l + b_l
    z_l   = segment_sum(msg, dst) / max(counts, 1)
Since segment-sum is linear and concat@W splits into blocks:
    z_l = (sum_m A@piece_m @ Wx_block_m  +  EA @ We_l + counts * b_l) / denom
where A is the (multi-)adjacency aggregation A@H[v] = sum_{e:dst=v} H[src_e],
EA = segment_sum(ea, dst), counts = in-degree.

Aggregation strategy (replaces per-edge dma_gather, which cost ~8.4ns/edge on
GpSimd): host builds each core's dense A^T as 79 chunks [128 src, 1280 dst]
(values = edge multiplicity); on device, aggT[ch, dst] accumulates
H_chunk^T @ A_chunk over chunks on the PE — H chunk [128 src, 128 ch] is the
stationary operand, A streams 1280 cols split into 3 PSUM-bank groups.
Last chunk has K=16 (10000 = 78*128 + 16), so no table padding is needed.

EA is aggregated once via per-tile one-hot matmuls (P built on-device from
offs/iota with is_equal; lhsT = [ea|1] tile), giving eaT [17, dst]; the
ones-column yields counts, and the bias is folded as wep row 16.  recip
(1/max(counts,1)) comes from the host — counts are pure edge-index data.

z_l per window: psum_z[dst, ch] = sum_m aggT_m^T @ Wx + eaT^T @ wep_l, then
a per-partition recip multiply.  AllGather (fp16) republishes each layer's z
as the next product's H table.  Edges sorted by dst; core c owns dst nodes
[1250c, 1250(c+1)).
"""
import os
import sys

sys.path.insert(0, "/opt/trn_rl_repo")

import numpy as np

_NPROD = int(os.environ.get("KERNEL_NPROD", "4"))  # debug knob: products to run
_NCC = int(os.environ.get("KERNEL_NCC", "3"))      # debug knob: collectives to run
_A_FP8 = os.environ.get("KERNEL_A_FP8", "1") == "1"  # A resident in fp8 vs fp16 stream

N_NODES = 10000
N_EDGES = 320000
CH = 128
EDGE_DIM = 16
EAD = EDGE_DIM + 1  # 17
EADP = 32  # EAD padded to the DVE 32x32 block-transpose granule
N_CORES = 8
NPC = N_NODES // N_CORES  # 1250 nodes per core
WPC = (NPC + 127) // 128  # 10 windows per core
DPC = WPC * 128  # 1280 padded dst cols per core
NCHUNK = (N_NODES + 127) // 128  # 79 src chunks (78 full + 16 rows)
LAST_ROWS = N_NODES - 128 * (NCHUNK - 1)  # 16
PAD_OFF = np.float16(255.0)  # offs value for padding edges (never matches iota)
COLGRP = [(0, 512), (512, 512), (1024, 256)]  # dst col groups (one PSUM bank each)


def _prep(edge_index, edge_attr):
    """Sort edges by dst; per core build dense A^T chunks, EA tile inputs
    (offs/ea padded to T tiles per window), and host-side recip."""
    src = np.asarray(edge_index[0], dtype=np.int64)
    dst = np.asarray(edge_index[1], dtype=np.int64)
    ea = np.asarray(edge_attr, dtype=np.float32)

    order = np.argsort(dst, kind="stable")
    src_s = src[order]
    dst_s = dst[order]
    ea_s = ea[order]

    # window boundaries: per core c, windows at nodes 1250c + 128w
    bounds = []
    for c in range(N_CORES):
        base = NPC * c
        for w in range(WPC):
            lo = base + 128 * w
            hi = min(base + 128 * (w + 1), base + NPC)
            bounds.append((lo, hi))
    starts = np.searchsorted(dst_s, [b[0] for b in bounds], side="left")
    ends = np.searchsorted(dst_s, [b[1] for b in bounds], side="left")
    counts_w = ends - starts
    T = max(1, int(np.max((counts_w + 127) // 128)))
    EPW = T * 128

    offs_all = np.full((N_CORES, WPC * EPW), PAD_OFF, np.float16)
    ea_all = np.zeros((N_CORES, WPC * EPW, EADP), np.float16)
    for bi, (lo, hi) in enumerate(bounds):
        c, w = divmod(bi, WPC)
        s, e = starts[bi], ends[bi]
        n = e - s
        o = w * EPW
        offs_all[c, o : o + n] = (dst_s[s:e] - lo).astype(np.float16)
        ea_all[c, o : o + n, :EDGE_DIM] = ea_s[s:e].astype(np.float16)
        ea_all[c, o : o + n, EDGE_DIM] = 1.0

    NT = WPC * T
    offs_pk = np.zeros((N_CORES, 128, NT), np.float16)
    ea_pk = np.zeros((N_CORES, 128, NT, EADP), np.float16)
    a_pk = np.zeros((N_CORES, NCHUNK, 128, DPC), np.float16)
    recip_pk = np.ones((N_CORES, 128, WPC), np.float32)
    for c in range(N_CORES):
        offs_pk[c] = offs_all[c].reshape(NT, 128).T
        ea_pk[c] = ea_all[c].reshape(NT, 128, EADP).transpose(1, 0, 2)
        s, e = starts[WPC * c], ends[WPC * c + WPC - 1]
        sl, dl = src_s[s:e], dst_s[s:e] - NPC * c
        flat = sl * DPC + dl  # src global row -> (chunk, row); dst -> col
        cnt = np.bincount(flat, minlength=N_NODES * DPC)
        a_pk[c] = _chunkify(cnt)
        dcnt = np.bincount(dl, minlength=DPC).astype(np.float32)
        recip_pk[c] = (1.0 / np.maximum(dcnt, 1.0)).reshape(WPC, 128).T
    return a_pk, offs_pk, ea_pk, recip_pk, T


def _chunkify(cnt):
    """[N_NODES*DPC] flat counts -> [NCHUNK, 128, DPC] with zero pad rows."""
    a = np.zeros((NCHUNK * 128, DPC), np.float16)
    a[:N_NODES] = cnt.reshape(-1, DPC)[:N_NODES].astype(np.float16)
    return a.reshape(NCHUNK, 128, DPC)


def _build(T, mybir, bass, tile, bacc):
    """Build the SPMD Bass program (same for all cores)."""
    fp16 = mybir.dt.float16
    f32 = mybir.dt.float32
    dt_a = mybir.dt.float8e4 if _A_FP8 else fp16
    NT = WPC * T

    nc = bacc.Bacc("TRN2", num_devices=N_CORES)
    x16 = nc.dram_tensor("x16", [N_NODES, CH], fp16, kind="ExternalInput")
    a_d = nc.dram_tensor("adj", [NCHUNK, 128, DPC], dt_a, kind="ExternalInput")
    offs_d = nc.dram_tensor("offs", [128, NT], fp16, kind="ExternalInput")
    ea_d = nc.dram_tensor("ea", [128, NT * EADP], fp16, kind="ExternalInput")
    wx_d = nc.dram_tensor("wx", [7, 128, 128], fp16, kind="ExternalInput")
    wep_d = nc.dram_tensor("wep", [4, EADP, 128], fp16, kind="ExternalInput")
    recip_d = nc.dram_tensor("recip", [128, WPC], f32, kind="ExternalInput")
    out_d = nc.dram_tensor("zout", [NPC, CH], f32, kind="ExternalOutput")

    # wx block index per (layer, piece): piece m aggregates table m
    # (0=x, 1=h0, 2=z1, 3=z2)
    PIECES = {0: [(0, 0)], 1: [(1, 1)], 2: [(1, 2), (2, 3)], 3: [(1, 4), (2, 5), (3, 6)]}
    wsizes = [128] * (WPC - 1) + [NPC - 128 * (WPC - 1)]

    with tile.TileContext(nc) as tc:
        with tc.tile_pool(name="singles", bufs=1) as singles, \
             tc.tile_pool(name="hpool", bufs=4) as hpool, \
             tc.tile_pool(name="apool", bufs=3) as apool, \
             tc.tile_pool(name="ppool", bufs=4) as ppool, \
             tc.tile_pool(name="zpool", bufs=2) as zpool, \
             tc.tile_pool(name="ps_agg", bufs=1, space="PSUM") as ps_agg, \
             tc.tile_pool(name="ps_ea", bufs=2, space="PSUM") as ps_ea, \
             tc.tile_pool(name="ps_z", bufs=2, space="PSUM") as ps_z, \
             tc.tile_pool(name="dram", bufs=1, space="DRAM") as dram:

            iota_t = singles.tile([128, 128], fp16)
            nc.gpsimd.iota(iota_t[:, :], pattern=[[1, 128]], channel_multiplier=0,
                           allow_small_or_imprecise_dtypes=True)
            wx_t = singles.tile([128, 7, 128], fp16)
            nc.sync.dma_start(out=wx_t[:, :, :], in_=wx_d[:, :, :].rearrange("k p j -> p k j"))
            wep_t = singles.tile([EADP, 4, 128], fp16)
            nc.sync.dma_start(out=wep_t[:, :, :], in_=wep_d[:, :, :].rearrange("l p j -> p l j"))
            offs_t = singles.tile([128, NT], fp16)
            nc.sync.dma_start(out=offs_t[:, :], in_=offs_d[:, :])
            ea_t = singles.tile([128, NT, EADP], fp16)
            nc.sync.dma_start(out=ea_t[:, :, :], in_=ea_d[:, :].rearrange("p (t j) -> p t j", j=EADP))
            recip_t = singles.tile([128, WPC], f32)
            nc.sync.dma_start(out=recip_t[:, :], in_=recip_d[:, :])

            if _A_FP8:
                a_t = singles.tile([128, NCHUNK, DPC], dt_a)
                nc.sync.dma_start(out=a_t[:, :, :],
                                  in_=a_d[:, :, :].rearrange("j p d -> p j d"))

            aggT_sb = singles.tile([128, 4, DPC], fp16)
            eaT_sb = singles.tile([EADP, WPC, 128], fp16)

            zin = [dram.tile([NPC, CH], fp16, name=f"zin{l}", tag=f"zin{l}") for l in range(3)]
            zfull = [dram.tile([N_NODES, CH], fp16, name=f"zfull{l}",
                               tag=f"zfull{l}", addr_space="Shared") for l in range(3)]

            # ---- EA pass (once): eaT[32, dst] per window -------------------
            # lhsT = P (one-hot, stationary), rhs = [ea|1] streamed (32 cols);
            # psum [dst, 32] is flipped to eaT [32, dst] by 4 DVE 32x32
            # block transposes.
            for w in range(WPC):
                psum_ea = ps_ea.tile([128, EADP], f32, tag="ea")
                for t in range(T):
                    tg = w * T + t
                    p_t = ppool.tile([128, 128], fp16, tag="p")
                    nc.vector.tensor_tensor(
                        out=p_t[:, :],
                        in0=offs_t[:, tg:tg + 1].to_broadcast([128, 128]),
                        in1=iota_t[:, :],
                        op=mybir.AluOpType.is_equal,
                    )
                    nc.tensor.matmul(psum_ea[:, :], lhsT=p_t[:, :], rhs=ea_t[:, tg, :],
                                     start=(t == 0), stop=(t == T - 1))
                ea_win = zpool.tile([128, EADP], fp16, tag="eawin")
                nc.vector.tensor_copy(out=ea_win[:, :], in_=psum_ea[:, :])
                for i in range(4):
                    nc.vector.transpose(out=eaT_sb[:, w, 32 * i:32 * i + 32],
                                        in_=ea_win[32 * i:32 * i + 32, :])

            # ---- products + layers ----------------------------------------
            for p in range(_NPROD):
                src_ap = x16 if p == 0 else zfull[p - 1]
                psg = [ps_agg.tile([128, cn], f32, name=f"agg{g}", tag=f"agg{g}")
                       for g, (c0, cn) in enumerate(COLGRP)]
                for j in range(NCHUNK):
                    rows = 128 if j < NCHUNK - 1 else LAST_ROWS
                    h = hpool.tile([128, 128], fp16, tag="h")
                    nc.sync.dma_start(out=h[:rows, :], in_=src_ap[128 * j:128 * j + rows, :])
                    if _A_FP8:
                        a_ap = a_t[:, j, :]
                    else:
                        a_st = apool.tile([128, DPC], dt_a, tag="a")
                        nc.sync.dma_start(out=a_st[:rows, :], in_=a_d[j, :rows, :])
                        a_ap = a_st[:, :]
                    for g, (c0, cn) in enumerate(COLGRP):
                        nc.tensor.matmul(psg[g][:, :], lhsT=h[:rows, :],
                                         rhs=a_ap[:rows, c0:c0 + cn],
                                         start=(j == 0), stop=(j == NCHUNK - 1))
                for g, (c0, cn) in enumerate(COLGRP):
                    nc.vector.tensor_copy(out=aggT_sb[:, p, c0:c0 + cn], in_=psg[g][:, :])

                for w in range(WPC):
                    psum_z = ps_z.tile([128, 128], f32, tag="z")
                    pieces = PIECES[p]
                    for i, (m, k) in enumerate(pieces):
                        nc.tensor.matmul(psum_z[:, :],
                                         lhsT=aggT_sb[:, m, 128 * w:128 * w + 128],
                                         rhs=wx_t[:, k, :], start=(i == 0), stop=False)
                    nc.tensor.matmul(psum_z[:, :], lhsT=eaT_sb[:, w, :],
                                     rhs=wep_t[:, p, :], start=False, stop=True)
                    wsz = wsizes[w]
                    if p < _NPROD - 1:
                        z_t = zpool.tile([128, 128], fp16, tag="z16")
                        nc.vector.tensor_scalar(
                            out=z_t[:, :], in0=psum_z[:, :],
                            scalar1=recip_t[:, w:w + 1], scalar2=None,
                            op0=mybir.AluOpType.mult,
                        )
                        nc.sync.dma_start(out=zin[p][128 * w:128 * w + wsz, :], in_=z_t[:wsz, :])
                    else:
                        z_t = zpool.tile([128, 128], f32, tag="z32")
                        nc.vector.tensor_scalar(
                            out=z_t[:, :], in0=psum_z[:, :],
                            scalar1=recip_t[:, w:w + 1], scalar2=None,
                            op0=mybir.AluOpType.mult,
                        )
                        nc.sync.dma_start(out=out_d[128 * w:128 * w + wsz, :], in_=z_t[:wsz, :])
                if p < _NCC and p < _NPROD - 1:
                    nc.gpsimd.collective_compute(
                        "AllGather", mybir.AluOpType.bypass,
                        replica_groups=[list(range(N_CORES))],
                        ins=[zin[p].opt()], outs=[zfull[p].opt()],
                    )
    nc.finalize()
    return nc


_CACHE = {}


def _get_program(T):
    key = (T, _A_FP8, _NPROD, _NCC)
    if key not in _CACHE:
        from concourse import mybir, bacc
        import concourse.bass as bass
        import concourse.tile as tile
        _CACHE[key] = _build(T, mybir, bass, tile, bacc)
    return _CACHE[key]


def _run(inputs, trace=False, tmpdir=None):
    from concourse.bass_utils import run_bass_kernel_spmd

    x = np.asarray(inputs["x"], np.float32)
    edge_attr = np.asarray(inputs["edge_attr"], np.float32)
    edge_index = np.asarray(inputs["edge_index"])
    Ws = [np.asarray(inputs[f"W{i}"], np.float32) for i in range(4)]
    bs = [np.asarray(inputs[f"b{i}"], np.float32) for i in range(4)]

    a_pk, offs_pk, ea_pk, recip_pk, T = _prep(edge_index, edge_attr)
    nc = _get_program(T)

    if _A_FP8:
        import ml_dtypes
        a_pk = a_pk.astype(ml_dtypes.float8_e4m3fn)

    x16 = x.astype(np.float16)
    # wx blocks: W0[:128], W1[:128], W2[:128], W2[128:256], W3[:128], W3[128:256], W3[256:384]
    wx = np.stack([
        Ws[0][:128], Ws[1][:128],
        Ws[2][:128], Ws[2][128:256],
        Ws[3][:128], Ws[3][128:256], Ws[3][256:384],
    ]).astype(np.float16)
    # wep: rows 0..15 = W_l[C_l:C_l+16], row 16 = b_l, rows 17..31 = 0
    Cs = [128, 128, 256, 384]
    wep = np.zeros((4, EADP, 128), np.float16)
    for l in range(4):
        wep[l, :EDGE_DIM] = Ws[l][Cs[l]:Cs[l] + EDGE_DIM].astype(np.float16)
        wep[l, EDGE_DIM] = bs[l].astype(np.float16)

    NT = WPC * T
    in_maps = []
    for c in range(N_CORES):
        in_maps.append({
            "x16": x16,
            "adj": a_pk[c],
            "offs": offs_pk[c],
            "ea": ea_pk[c].reshape(128, NT * EADP),
            "wx": wx,
            "wep": wep,
            "recip": recip_pk[c],
        })
    res = run_bass_kernel_spmd(nc, in_maps, core_ids=list(range(N_CORES)),
                               trace=trace, tmpdir=tmpdir)
    out = np.concatenate([res.results[c]["zout"] for c in range(N_CORES)], axis=0)
    return out, res


def kernel(**inputs) -> np.ndarray:
    out, _ = _run(inputs, trace=False)
    return out


# revision 13
# speedup vs baseline: 1.1388x; 1.1388x over previous
"""DenseGCNBlock on 8 trn2 NeuronCores (Bass/Tile) — dense-adjacency version.

Math: reference computes, per layer l with weight W_l [C_l+16, 128]:
    msg_e = concat(cat[src_e], ea_e) @ W_l + b_l
    z_l   = segment_sum(msg, dst) / max(counts, 1)
Since segment-sum is linear and concat@W splits into blocks:
    z_l = (sum_m A@piece_m @ Wx_block_m  +  EA @ We_l + counts * b_l) / denom
where A is the (multi-)adjacency aggregation A@H[v] = sum_{e:dst=v} H[src_e],
EA = segment_sum(ea, dst), counts = in-degree.

Aggregation strategy (replaces per-edge dma_gather, which cost ~8.4ns/edge on
GpSimd): host builds each core's dense A^T as 79 chunks [128 src, 1280 dst]
(values = edge multiplicity); on device, aggT[ch, dst] accumulates
H_chunk^T @ A_chunk over chunks on the PE — H chunk [128 src, 128 ch] is the
stationary operand, A streams 1280 cols split into 3 PSUM-bank groups.
Last chunk has K=16 (10000 = 78*128 + 16), so no table padding is needed.

EA is aggregated once via per-tile one-hot matmuls (P built on-device from
offs/iota with is_equal; lhsT = [ea|1] tile), giving eaT [17, dst]; the
ones-column yields counts, and the bias is folded as wep row 16.  recip
(1/max(counts,1)) comes from the host — counts are pure edge-index data.

z_l per window: psum_z[dst, ch] = sum_m aggT_m^T @ Wx + eaT^T @ wep_l, then
a per-partition recip multiply.  AllGather (fp16) republishes each layer's z
as the next product's H table.  Edges sorted by dst; core c owns dst nodes
[1250c, 1250(c+1)).
"""
import os
import sys

sys.path.insert(0, "/opt/trn_rl_repo")

import numpy as np

_NPROD = int(os.environ.get("KERNEL_NPROD", "4"))  # debug knob: products to run
_NCC = int(os.environ.get("KERNEL_NCC", "3"))      # debug knob: collectives to run
_A_FP8 = os.environ.get("KERNEL_A_FP8", "1") == "1"  # A resident in fp8 vs fp16 stream

N_NODES = 10000
N_EDGES = 320000
CH = 128
EDGE_DIM = 16
EAD = EDGE_DIM + 1  # 17
EADP = 32  # EAD padded to the DVE 32x32 block-transpose granule
N_CORES = 8
NPC = N_NODES // N_CORES  # 1250 nodes per core
WPC = (NPC + 127) // 128  # 10 windows per core
DPC = WPC * 128  # 1280 padded dst cols per core
NCHUNK = (N_NODES + 127) // 128  # 79 src chunks (78 full + 16 rows)
LAST_ROWS = N_NODES - 128 * (NCHUNK - 1)  # 16
PAD_OFF = np.float16(255.0)  # offs value for padding edges (never matches iota)
COLGRP = [(0, 512), (512, 512), (1024, 256)]  # dst col groups (one PSUM bank each)


def _prep(edge_index, edge_attr):
    """Sort edges by dst; per core build dense A^T chunks, EA tile inputs
    (offs/ea padded to T tiles per window), and host-side recip."""
    src = np.asarray(edge_index[0], dtype=np.int64)
    dst = np.asarray(edge_index[1], dtype=np.int64)
    ea = np.asarray(edge_attr, dtype=np.float32)

    order = np.argsort(dst, kind="stable")
    src_s = src[order]
    dst_s = dst[order]
    ea_s = ea[order]

    # window boundaries: per core c, windows at nodes 1250c + 128w
    bounds = []
    for c in range(N_CORES):
        base = NPC * c
        for w in range(WPC):
            lo = base + 128 * w
            hi = min(base + 128 * (w + 1), base + NPC)
            bounds.append((lo, hi))
    starts = np.searchsorted(dst_s, [b[0] for b in bounds], side="left")
    ends = np.searchsorted(dst_s, [b[1] for b in bounds], side="left")
    counts_w = ends - starts
    T = max(1, int(np.max((counts_w + 127) // 128)))
    EPW = T * 128

    offs_all = np.full((N_CORES, WPC * EPW), PAD_OFF, np.float16)
    ea_all = np.zeros((N_CORES, WPC * EPW, EADP), np.float16)
    for bi, (lo, hi) in enumerate(bounds):
        c, w = divmod(bi, WPC)
        s, e = starts[bi], ends[bi]
        n = e - s
        o = w * EPW
        offs_all[c, o : o + n] = (dst_s[s:e] - lo).astype(np.float16)
        ea_all[c, o : o + n, :EDGE_DIM] = ea_s[s:e].astype(np.float16)
        ea_all[c, o : o + n, EDGE_DIM] = 1.0

    NT = WPC * T
    offs_pk = np.zeros((N_CORES, 128, NT), np.float16)
    ea_pk = np.zeros((N_CORES, 128, NT, EADP), np.float16)
    a_pk = np.zeros((N_CORES, NCHUNK, 128, DPC), np.float16)
    recip_pk = np.ones((N_CORES, 128, WPC), np.float32)
    for c in range(N_CORES):
        offs_pk[c] = offs_all[c].reshape(NT, 128).T
        ea_pk[c] = ea_all[c].reshape(NT, 128, EADP).transpose(1, 0, 2)
        s, e = starts[WPC * c], ends[WPC * c + WPC - 1]
        sl, dl = src_s[s:e], dst_s[s:e] - NPC * c
        flat = sl * DPC + dl  # src global row -> (chunk, row); dst -> col
        cnt = np.bincount(flat, minlength=N_NODES * DPC)
        a_pk[c] = _chunkify(cnt)
        dcnt = np.bincount(dl, minlength=DPC).astype(np.float32)
        recip_pk[c] = (1.0 / np.maximum(dcnt, 1.0)).reshape(WPC, 128).T
    return a_pk, offs_pk, ea_pk, recip_pk, T


def _chunkify(cnt):
    """[N_NODES*DPC] flat counts -> [NCHUNK, 128, DPC] with zero pad rows."""
    a = np.zeros((NCHUNK * 128, DPC), np.float16)
    a[:N_NODES] = cnt.reshape(-1, DPC)[:N_NODES].astype(np.float16)
    return a.reshape(NCHUNK, 128, DPC)


def _build(T, mybir, bass, tile, bacc):
    """Build the SPMD Bass program (same for all cores)."""
    fp16 = mybir.dt.float16
    f32 = mybir.dt.float32
    dt_a = mybir.dt.float8e4 if _A_FP8 else fp16
    NT = WPC * T

    nc = bacc.Bacc("TRN2", num_devices=N_CORES)
    x16 = nc.dram_tensor("x16", [N_NODES, CH], fp16, kind="ExternalInput")
    a_d = nc.dram_tensor("adj", [NCHUNK, 128, DPC], dt_a, kind="ExternalInput")
    offs_d = nc.dram_tensor("offs", [128, NT], fp16, kind="ExternalInput")
    ea_d = nc.dram_tensor("ea", [128, NT * EADP], fp16, kind="ExternalInput")
    wx_d = nc.dram_tensor("wx", [7, 128, 128], fp16, kind="ExternalInput")
    wep_d = nc.dram_tensor("wep", [4, EADP, 128], fp16, kind="ExternalInput")
    recip_d = nc.dram_tensor("recip", [128, WPC], f32, kind="ExternalInput")
    out_d = nc.dram_tensor("zout", [NPC, CH], f32, kind="ExternalOutput")

    # wx block index per (layer, piece): piece m aggregates table m
    # (0=x, 1=h0, 2=z1, 3=z2)
    PIECES = {0: [(0, 0)], 1: [(1, 1)], 2: [(1, 2), (2, 3)], 3: [(1, 4), (2, 5), (3, 6)]}
    wsizes = [128] * (WPC - 1) + [NPC - 128 * (WPC - 1)]

    with tile.TileContext(nc) as tc:
        with tc.tile_pool(name="singles", bufs=1) as singles, \
             tc.tile_pool(name="hpool", bufs=4) as hpool, \
             tc.tile_pool(name="apool", bufs=3) as apool, \
             tc.tile_pool(name="ppool", bufs=4) as ppool, \
             tc.tile_pool(name="zpool", bufs=2) as zpool, \
             tc.tile_pool(name="ps_agg", bufs=1, space="PSUM") as ps_agg, \
             tc.tile_pool(name="ps_ea", bufs=2, space="PSUM") as ps_ea, \
             tc.tile_pool(name="ps_z", bufs=2, space="PSUM") as ps_z, \
             tc.tile_pool(name="dram", bufs=1, space="DRAM") as dram:

            iota_t = singles.tile([128, 128], fp16)
            nc.gpsimd.iota(iota_t[:, :], pattern=[[1, 128]], channel_multiplier=0,
                           allow_small_or_imprecise_dtypes=True)
            wx_t = singles.tile([128, 7, 128], fp16)
            nc.sync.dma_start(out=wx_t[:, :, :], in_=wx_d[:, :, :].rearrange("k p j -> p k j"))
            wep_t = singles.tile([EADP, 4, 128], fp16)
            nc.sync.dma_start(out=wep_t[:, :, :], in_=wep_d[:, :, :].rearrange("l p j -> p l j"))
            offs_t = singles.tile([128, NT], fp16)
            nc.sync.dma_start(out=offs_t[:, :], in_=offs_d[:, :])
            ea_t = singles.tile([128, NT, EADP], fp16)
            nc.sync.dma_start(out=ea_t[:, :, :], in_=ea_d[:, :].rearrange("p (t j) -> p t j", j=EADP))
            recip_t = singles.tile([128, WPC], f32)
            nc.sync.dma_start(out=recip_t[:, :], in_=recip_d[:, :])

            if _A_FP8:
                a_t = singles.tile([128, NCHUNK, DPC], dt_a)
                nc.sync.dma_start(out=a_t[:, :, :],
                                  in_=a_d[:, :, :].rearrange("j p d -> p j d"))

            aggT_sb = singles.tile([128, 4, DPC], fp16)
            eaT_sb = singles.tile([EADP, WPC, 128], fp16)

            zin = [dram.tile([NPC, CH], fp16, name=f"zin{l}", tag=f"zin{l}") for l in range(3)]
            zfull = [dram.tile([N_NODES, CH], fp16, name=f"zfull{l}", tag=f"zfull{l}") for l in range(3)]

            # ---- EA pass (once): eaT[32, dst] per window -------------------
            # lhsT = P (one-hot, stationary), rhs = [ea|1] streamed (32 cols);
            # psum [dst, 32] is flipped to eaT [32, dst] by 4 DVE 32x32
            # block transposes.
            for w in range(WPC):
                psum_ea = ps_ea.tile([128, EADP], f32, tag="ea")
                for t in range(T):
                    tg = w * T + t
                    p_t = ppool.tile([128, 128], fp16, tag="p")
                    nc.vector.tensor_tensor(
                        out=p_t[:, :],
                        in0=offs_t[:, tg:tg + 1].to_broadcast([128, 128]),
                        in1=iota_t[:, :],
                        op=mybir.AluOpType.is_equal,
                    )
                    nc.tensor.matmul(psum_ea[:, :], lhsT=p_t[:, :], rhs=ea_t[:, tg, :],
                                     start=(t == 0), stop=(t == T - 1))
                ea_win = zpool.tile([128, EADP], fp16, tag="eawin")
                nc.vector.tensor_copy(out=ea_win[:, :], in_=psum_ea[:, :])
                for i in range(4):
                    nc.vector.transpose(out=eaT_sb[:, w, 32 * i:32 * i + 32],
                                        in_=ea_win[32 * i:32 * i + 32, :])

            # ---- products + layers ----------------------------------------
            for p in range(_NPROD):
                src_ap = x16 if p == 0 else zfull[p - 1]
                psg = [ps_agg.tile([128, cn], f32, name=f"agg{g}", tag=f"agg{g}")
                       for g, (c0, cn) in enumerate(COLGRP)]
                for j in range(NCHUNK):
                    rows = 128 if j < NCHUNK - 1 else LAST_ROWS
                    h = hpool.tile([128, 128], fp16, tag="h")
                    nc.sync.dma_start(out=h[:rows, :], in_=src_ap[128 * j:128 * j + rows, :])
                    if _A_FP8:
                        a_ap = a_t[:, j, :]
                    else:
                        a_st = apool.tile([128, DPC], dt_a, tag="a")
                        nc.sync.dma_start(out=a_st[:rows, :], in_=a_d[j, :rows, :])
                        a_ap = a_st[:, :]
                    for g, (c0, cn) in enumerate(COLGRP):
                        nc.tensor.matmul(psg[g][:, :], lhsT=h[:rows, :],
                                         rhs=a_ap[:rows, c0:c0 + cn],
                                         start=(j == 0), stop=(j == NCHUNK - 1))
                for g, (c0, cn) in enumerate(COLGRP):
                    nc.vector.tensor_copy(out=aggT_sb[:, p, c0:c0 + cn], in_=psg[g][:, :])

                for w in range(WPC):
                    psum_z = ps_z.tile([128, 128], f32, tag="z")
                    pieces = PIECES[p]
                    for i, (m, k) in enumerate(pieces):
                        nc.tensor.matmul(psum_z[:, :],
                                         lhsT=aggT_sb[:, m, 128 * w:128 * w + 128],
                                         rhs=wx_t[:, k, :], start=(i == 0), stop=False)
                    nc.tensor.matmul(psum_z[:, :], lhsT=eaT_sb[:, w, :],
                                     rhs=wep_t[:, p, :], start=False, stop=True)
                    wsz = wsizes[w]
                    if p < _NPROD - 1:
                        z_t = zpool.tile([128, 128], fp16, tag="z16")
                        nc.vector.tensor_scalar(
                            out=z_t[:, :], in0=psum_z[:, :],
                            scalar1=recip_t[:, w:w + 1], scalar2=None,
                            op0=mybir.AluOpType.mult,
                        )
                        nc.sync.dma_start(out=zin[p][128 * w:128 * w + wsz, :], in_=z_t[:wsz, :])
                    else:
                        z_t = zpool.tile([128, 128], f32, tag="z32")
                        nc.vector.tensor_scalar(
                            out=z_t[:, :], in0=psum_z[:, :],
                            scalar1=recip_t[:, w:w + 1], scalar2=None,
                            op0=mybir.AluOpType.mult,
                        )
                        nc.sync.dma_start(out=out_d[128 * w:128 * w + wsz, :], in_=z_t[:wsz, :])
                if p < _NCC and p < _NPROD - 1:
                    nc.gpsimd.collective_compute(
                        "AllGather", mybir.AluOpType.bypass,
                        replica_groups=[list(range(N_CORES))],
                        ins=[zin[p].opt()], outs=[zfull[p].opt()],
                    )
    nc.finalize()
    return nc


_CACHE = {}


def _get_program(T):
    key = (T, _A_FP8, _NPROD, _NCC)
    if key not in _CACHE:
        from concourse import mybir, bacc
        import concourse.bass as bass
        import concourse.tile as tile
        _CACHE[key] = _build(T, mybir, bass, tile, bacc)
    return _CACHE[key]


def _run(inputs, trace=False, tmpdir=None):
    from concourse.bass_utils import run_bass_kernel_spmd

    x = np.asarray(inputs["x"], np.float32)
    edge_attr = np.asarray(inputs["edge_attr"], np.float32)
    edge_index = np.asarray(inputs["edge_index"])
    Ws = [np.asarray(inputs[f"W{i}"], np.float32) for i in range(4)]
    bs = [np.asarray(inputs[f"b{i}"], np.float32) for i in range(4)]

    a_pk, offs_pk, ea_pk, recip_pk, T = _prep(edge_index, edge_attr)
    nc = _get_program(T)

    if _A_FP8:
        import ml_dtypes
        a_pk = a_pk.astype(ml_dtypes.float8_e4m3fn)

    x16 = x.astype(np.float16)
    # wx blocks: W0[:128], W1[:128], W2[:128], W2[128:256], W3[:128], W3[128:256], W3[256:384]
    wx = np.stack([
        Ws[0][:128], Ws[1][:128],
        Ws[2][:128], Ws[2][128:256],
        Ws[3][:128], Ws[3][128:256], Ws[3][256:384],
    ]).astype(np.float16)
    # wep: rows 0..15 = W_l[C_l:C_l+16], row 16 = b_l, rows 17..31 = 0
    Cs = [128, 128, 256, 384]
    wep = np.zeros((4, EADP, 128), np.float16)
    for l in range(4):
        wep[l, :EDGE_DIM] = Ws[l][Cs[l]:Cs[l] + EDGE_DIM].astype(np.float16)
        wep[l, EDGE_DIM] = bs[l].astype(np.float16)

    NT = WPC * T
    in_maps = []
    for c in range(N_CORES):
        in_maps.append({
            "x16": x16,
            "adj": a_pk[c],
            "offs": offs_pk[c],
            "ea": ea_pk[c].reshape(128, NT * EADP),
            "wx": wx,
            "wep": wep,
            "recip": recip_pk[c],
        })
    res = run_bass_kernel_spmd(nc, in_maps, core_ids=list(range(N_CORES)),
                               trace=trace, tmpdir=tmpdir)
    out = np.concatenate([res.results[c]["zout"] for c in range(N_CORES)], axis=0)
    return out, res


def kernel(**inputs) -> np.ndarray:
    out, _ = _run(inputs, trace=False)
    return out


# revision 14
# speedup vs baseline: 1.1524x; 1.0120x over previous
"""DenseGCNBlock on 8 trn2 NeuronCores (Bass/Tile) — dense-adjacency version.

Math: reference computes, per layer l with weight W_l [C_l+16, 128]:
    msg_e = concat(cat[src_e], ea_e) @ W_l + b_l
    z_l   = segment_sum(msg, dst) / max(counts, 1)
Since segment-sum is linear and concat@W splits into blocks:
    z_l = (sum_m A@piece_m @ Wx_block_m  +  EA @ We_l + counts * b_l) / denom
where A is the (multi-)adjacency aggregation A@H[v] = sum_{e:dst=v} H[src_e],
EA = segment_sum(ea, dst), counts = in-degree.

Aggregation strategy (replaces per-edge dma_gather, which cost ~8.4ns/edge on
GpSimd): host builds each core's dense A^T as 79 chunks [128 src, 1280 dst]
(values = edge multiplicity); on device, aggT[ch, dst] accumulates
H_chunk^T @ A_chunk over chunks on the PE — H chunk [128 src, 128 ch] is the
stationary operand, A streams 1280 cols split into 3 PSUM-bank groups.
Last chunk has K=16 (10000 = 78*128 + 16), so no table padding is needed.

EA is aggregated once via per-tile one-hot matmuls (P built on-device from
offs/iota with is_equal; lhsT = [ea|1] tile), giving eaT [17, dst]; the
ones-column yields counts, and the bias is folded as wep row 16.  recip
(1/max(counts,1)) comes from the host — counts are pure edge-index data.

z_l per window: psum_z[dst, ch] = sum_m aggT_m^T @ Wx + eaT^T @ wep_l, then
a per-partition recip multiply.  AllGather (fp16) republishes each layer's z
as the next product's H table.  Edges sorted by dst; core c owns dst nodes
[1250c, 1250(c+1)).
"""
import os
import sys

sys.path.insert(0, "/opt/trn_rl_repo")

import numpy as np

_NPROD = int(os.environ.get("KERNEL_NPROD", "4"))  # debug knob: products to run
_NCC = int(os.environ.get("KERNEL_NCC", "3"))      # debug knob: collectives to run
_A_FP8 = os.environ.get("KERNEL_A_FP8", "1") == "1"  # A resident in fp8 vs fp16 stream

N_NODES = 10000
N_EDGES = 320000
CH = 128
EDGE_DIM = 16
EAD = EDGE_DIM + 1  # 17
EADP = 32  # EAD padded to the DVE 32x32 block-transpose granule
N_CORES = 8
NPC = N_NODES // N_CORES  # 1250 nodes per core
WPC = (NPC + 127) // 128  # 10 windows per core
DPC = WPC * 128  # 1280 padded dst cols per core
NCHUNK = (N_NODES + 127) // 128  # 79 src chunks (78 full + 16 rows)
LAST_ROWS = N_NODES - 128 * (NCHUNK - 1)  # 16
PAD_OFF = np.float16(255.0)  # offs value for padding edges (never matches iota)
COLGRP = [(0, 512), (512, 512), (1024, 256)]  # dst col groups (one PSUM bank each)


def _prep(edge_index, edge_attr):
    """Sort edges by dst; per core build dense A^T chunks, EA tile inputs
    (offs/ea padded to T tiles per window), and host-side recip."""
    src = np.asarray(edge_index[0], dtype=np.int64)
    dst = np.asarray(edge_index[1], dtype=np.int64)
    ea = np.asarray(edge_attr, dtype=np.float32)

    order = np.argsort(dst, kind="stable")
    src_s = src[order]
    dst_s = dst[order]
    ea_s = ea[order]

    # window boundaries: per core c, windows at nodes 1250c + 128w
    bounds = []
    for c in range(N_CORES):
        base = NPC * c
        for w in range(WPC):
            lo = base + 128 * w
            hi = min(base + 128 * (w + 1), base + NPC)
            bounds.append((lo, hi))
    starts = np.searchsorted(dst_s, [b[0] for b in bounds], side="left")
    ends = np.searchsorted(dst_s, [b[1] for b in bounds], side="left")
    counts_w = ends - starts
    T = max(1, int(np.max((counts_w + 127) // 128)))
    EPW = T * 128

    offs_all = np.full((N_CORES, WPC * EPW), PAD_OFF, np.float16)
    ea_all = np.zeros((N_CORES, WPC * EPW, EADP), np.float16)
    for bi, (lo, hi) in enumerate(bounds):
        c, w = divmod(bi, WPC)
        s, e = starts[bi], ends[bi]
        n = e - s
        o = w * EPW
        offs_all[c, o : o + n] = (dst_s[s:e] - lo).astype(np.float16)
        ea_all[c, o : o + n, :EDGE_DIM] = ea_s[s:e].astype(np.float16)
        ea_all[c, o : o + n, EDGE_DIM] = 1.0

    NT = WPC * T
    offs_pk = np.zeros((N_CORES, 128, NT), np.float16)
    ea_pk = np.zeros((N_CORES, 128, NT, EADP), np.float16)
    a_pk = np.zeros((N_CORES, NCHUNK, 128, DPC), np.float16)
    recip_pk = np.ones((N_CORES, 128, WPC), np.float32)
    for c in range(N_CORES):
        offs_pk[c] = offs_all[c].reshape(NT, 128).T
        ea_pk[c] = ea_all[c].reshape(NT, 128, EADP).transpose(1, 0, 2)
        s, e = starts[WPC * c], ends[WPC * c + WPC - 1]
        sl, dl = src_s[s:e], dst_s[s:e] - NPC * c
        flat = sl * DPC + dl  # src global row -> (chunk, row); dst -> col
        cnt = np.bincount(flat, minlength=N_NODES * DPC)
        a_pk[c] = _chunkify(cnt)
        dcnt = np.bincount(dl, minlength=DPC).astype(np.float32)
        recip_pk[c] = (1.0 / np.maximum(dcnt, 1.0)).reshape(WPC, 128).T
    return a_pk, offs_pk, ea_pk, recip_pk, T


def _chunkify(cnt):
    """[N_NODES*DPC] flat counts -> [NCHUNK, 128, DPC] with zero pad rows."""
    a = np.zeros((NCHUNK * 128, DPC), np.float16)
    a[:N_NODES] = cnt.reshape(-1, DPC)[:N_NODES].astype(np.float16)
    return a.reshape(NCHUNK, 128, DPC)


def _build(T, mybir, bass, tile, bacc):
    """Build the SPMD Bass program (same for all cores)."""
    fp16 = mybir.dt.float16
    f32 = mybir.dt.float32
    dt_a = mybir.dt.float8e4 if _A_FP8 else fp16
    NT = WPC * T

    nc = bacc.Bacc("TRN2", num_devices=N_CORES)
    x16 = nc.dram_tensor("x16", [N_NODES, CH], fp16, kind="ExternalInput")
    a_d = nc.dram_tensor("adj", [NCHUNK, 128, DPC], dt_a, kind="ExternalInput")
    offs_d = nc.dram_tensor("offs", [128, NT], fp16, kind="ExternalInput")
    ea_d = nc.dram_tensor("ea", [128, NT * EADP], fp16, kind="ExternalInput")
    wx_d = nc.dram_tensor("wx", [7, 128, 128], fp16, kind="ExternalInput")
    wep_d = nc.dram_tensor("wep", [4, EADP, 128], fp16, kind="ExternalInput")
    recip_d = nc.dram_tensor("recip", [128, WPC], f32, kind="ExternalInput")
    out_d = nc.dram_tensor("zout", [NPC, CH], f32, kind="ExternalOutput")

    # wx block index per (layer, piece): piece m aggregates table m
    # (0=x, 1=h0, 2=z1, 3=z2)
    PIECES = {0: [(0, 0)], 1: [(1, 1)], 2: [(1, 2), (2, 3)], 3: [(1, 4), (2, 5), (3, 6)]}
    wsizes = [128] * (WPC - 1) + [NPC - 128 * (WPC - 1)]

    with tile.TileContext(nc) as tc:
        with tc.tile_pool(name="singles", bufs=1) as singles, \
             tc.tile_pool(name="hpool", bufs=4) as hpool, \
             tc.tile_pool(name="apool", bufs=3) as apool, \
             tc.tile_pool(name="ppool", bufs=4) as ppool, \
             tc.tile_pool(name="zpool", bufs=2) as zpool, \
             tc.tile_pool(name="ps_agg", bufs=1, space="PSUM") as ps_agg, \
             tc.tile_pool(name="ps_ea", bufs=2, space="PSUM") as ps_ea, \
             tc.tile_pool(name="ps_z", bufs=2, space="PSUM") as ps_z, \
             tc.tile_pool(name="dram", bufs=1, space="DRAM") as dram:

            iota_t = singles.tile([128, 128], fp16)
            nc.gpsimd.iota(iota_t[:, :], pattern=[[1, 128]], channel_multiplier=0,
                           allow_small_or_imprecise_dtypes=True)
            wx_t = singles.tile([128, 7, 128], fp16)
            nc.sync.dma_start(out=wx_t[:, :, :], in_=wx_d[:, :, :].rearrange("k p j -> p k j"))
            wep_t = singles.tile([EADP, 4, 128], fp16)
            nc.sync.dma_start(out=wep_t[:, :, :], in_=wep_d[:, :, :].rearrange("l p j -> p l j"))
            offs_t = singles.tile([128, NT], fp16)
            nc.sync.dma_start(out=offs_t[:, :], in_=offs_d[:, :])
            ea_t = singles.tile([128, NT, EADP], fp16)
            nc.sync.dma_start(out=ea_t[:, :, :], in_=ea_d[:, :].rearrange("p (t j) -> p t j", j=EADP))
            recip_t = singles.tile([128, WPC], f32)
            nc.sync.dma_start(out=recip_t[:, :], in_=recip_d[:, :])

            if _A_FP8:
                a_t = singles.tile([128, NCHUNK, DPC], dt_a)
                nc.sync.dma_start(out=a_t[:, :, :],
                                  in_=a_d[:, :, :].rearrange("j p d -> p j d"))

            aggT_sb = singles.tile([128, 4, DPC], fp16)
            eaT_sb = singles.tile([EADP, WPC, 128], fp16)

            zin = [dram.tile([NPC, CH], fp16, name=f"zin{l}", tag=f"zin{l}") for l in range(3)]
            zfull = [dram.tile([N_NODES, CH], fp16, name=f"zfull{l}", tag=f"zfull{l}") for l in range(3)]

            # tiny warmup AllGather: absorbs the first-collective RDH setup
            # cost (~30us) under the EA/product-0 compute
            warm_in = dram.tile([8, CH], fp16, name="warm_in", tag="warm_in")
            warm_out = dram.tile([64, CH], fp16, name="warm_out", tag="warm_out")
            wz = zpool.tile([8, CH], fp16, tag="wz")
            nc.vector.memset(wz[:, :], 0.0)
            nc.sync.dma_start(out=warm_in[:, :], in_=wz[:, :])
            nc.gpsimd.collective_compute(
                "AllGather", mybir.AluOpType.bypass,
                replica_groups=[list(range(N_CORES))],
                ins=[warm_in.opt()], outs=[warm_out.opt()],
            )

            # ---- EA pass (once): eaT[32, dst] per window -------------------
            # lhsT = P (one-hot, stationary), rhs = [ea|1] streamed (32 cols);
            # psum [dst, 32] is flipped to eaT [32, dst] by 4 DVE 32x32
            # block transposes.
            for w in range(WPC):
                psum_ea = ps_ea.tile([128, EADP], f32, tag="ea")
                for t in range(T):
                    tg = w * T + t
                    p_t = ppool.tile([128, 128], fp16, tag="p")
                    nc.vector.tensor_tensor(
                        out=p_t[:, :],
                        in0=offs_t[:, tg:tg + 1].to_broadcast([128, 128]),
                        in1=iota_t[:, :],
                        op=mybir.AluOpType.is_equal,
                    )
                    nc.tensor.matmul(psum_ea[:, :], lhsT=p_t[:, :], rhs=ea_t[:, tg, :],
                                     start=(t == 0), stop=(t == T - 1))
                ea_win = zpool.tile([128, EADP], fp16, tag="eawin")
                nc.vector.tensor_copy(out=ea_win[:, :], in_=psum_ea[:, :])
                for i in range(4):
                    nc.vector.transpose(out=eaT_sb[:, w, 32 * i:32 * i + 32],
                                        in_=ea_win[32 * i:32 * i + 32, :])

            # ---- products + layers ----------------------------------------
            for p in range(_NPROD):
                src_ap = x16 if p == 0 else zfull[p - 1]
                psg = [ps_agg.tile([128, cn], f32, name=f"agg{g}", tag=f"agg{g}")
                       for g, (c0, cn) in enumerate(COLGRP)]
                for j in range(NCHUNK):
                    rows = 128 if j < NCHUNK - 1 else LAST_ROWS
                    h = hpool.tile([128, 128], fp16, tag="h")
                    nc.sync.dma_start(out=h[:rows, :], in_=src_ap[128 * j:128 * j + rows, :])
                    if _A_FP8:
                        a_ap = a_t[:, j, :]
                    else:
                        a_st = apool.tile([128, DPC], dt_a, tag="a")
                        nc.sync.dma_start(out=a_st[:rows, :], in_=a_d[j, :rows, :])
                        a_ap = a_st[:, :]
                    for g, (c0, cn) in enumerate(COLGRP):
                        nc.tensor.matmul(psg[g][:, :], lhsT=h[:rows, :],
                                         rhs=a_ap[:rows, c0:c0 + cn],
                                         start=(j == 0), stop=(j == NCHUNK - 1))
                for g, (c0, cn) in enumerate(COLGRP):
                    nc.vector.tensor_copy(out=aggT_sb[:, p, c0:c0 + cn], in_=psg[g][:, :])

                for w in range(WPC):
                    psum_z = ps_z.tile([128, 128], f32, tag="z")
                    pieces = PIECES[p]
                    for i, (m, k) in enumerate(pieces):
                        nc.tensor.matmul(psum_z[:, :],
                                         lhsT=aggT_sb[:, m, 128 * w:128 * w + 128],
                                         rhs=wx_t[:, k, :], start=(i == 0), stop=False)
                    nc.tensor.matmul(psum_z[:, :], lhsT=eaT_sb[:, w, :],
                                     rhs=wep_t[:, p, :], start=False, stop=True)
                    wsz = wsizes[w]
                    if p < _NPROD - 1:
                        z_t = zpool.tile([128, 128], fp16, tag="z16")
                        nc.vector.tensor_scalar(
                            out=z_t[:, :], in0=psum_z[:, :],
                            scalar1=recip_t[:, w:w + 1], scalar2=None,
                            op0=mybir.AluOpType.mult,
                        )
                        nc.sync.dma_start(out=zin[p][128 * w:128 * w + wsz, :], in_=z_t[:wsz, :])
                    else:
                        z_t = zpool.tile([128, 128], f32, tag="z32")
                        nc.vector.tensor_scalar(
                            out=z_t[:, :], in0=psum_z[:, :],
                            scalar1=recip_t[:, w:w + 1], scalar2=None,
                            op0=mybir.AluOpType.mult,
                        )
                        nc.sync.dma_start(out=out_d[128 * w:128 * w + wsz, :], in_=z_t[:wsz, :])
                if p < _NCC and p < _NPROD - 1:
                    nc.gpsimd.collective_compute(
                        "AllGather", mybir.AluOpType.bypass,
                        replica_groups=[list(range(N_CORES))],
                        ins=[zin[p].opt()], outs=[zfull[p].opt()],
                    )
    nc.finalize()
    return nc


_CACHE = {}


def _get_program(T):
    key = (T, _A_FP8, _NPROD, _NCC)
    if key not in _CACHE:
        from concourse import mybir, bacc
        import concourse.bass as bass
        import concourse.tile as tile
        _CACHE[key] = _build(T, mybir, bass, tile, bacc)
    return _CACHE[key]


def _run(inputs, trace=False, tmpdir=None):
    from concourse.bass_utils import run_bass_kernel_spmd

    x = np.asarray(inputs["x"], np.float32)
    edge_attr = np.asarray(inputs["edge_attr"], np.float32)
    edge_index = np.asarray(inputs["edge_index"])
    Ws = [np.asarray(inputs[f"W{i}"], np.float32) for i in range(4)]
    bs = [np.asarray(inputs[f"b{i}"], np.float32) for i in range(4)]

    a_pk, offs_pk, ea_pk, recip_pk, T = _prep(edge_index, edge_attr)
    nc = _get_program(T)

    if _A_FP8:
        import ml_dtypes
        a_pk = a_pk.astype(ml_dtypes.float8_e4m3fn)

    x16 = x.astype(np.float16)
    # wx blocks: W0[:128], W1[:128], W2[:128], W2[128:256], W3[:128], W3[128:256], W3[256:384]
    wx = np.stack([
        Ws[0][:128], Ws[1][:128],
        Ws[2][:128], Ws[2][128:256],
        Ws[3][:128], Ws[3][128:256], Ws[3][256:384],
    ]).astype(np.float16)
    # wep: rows 0..15 = W_l[C_l:C_l+16], row 16 = b_l, rows 17..31 = 0
    Cs = [128, 128, 256, 384]
    wep = np.zeros((4, EADP, 128), np.float16)
    for l in range(4):
        wep[l, :EDGE_DIM] = Ws[l][Cs[l]:Cs[l] + EDGE_DIM].astype(np.float16)
        wep[l, EDGE_DIM] = bs[l].astype(np.float16)

    NT = WPC * T
    in_maps = []
    for c in range(N_CORES):
        in_maps.append({
            "x16": x16,
            "adj": a_pk[c],
            "offs": offs_pk[c],
            "ea": ea_pk[c].reshape(128, NT * EADP),
            "wx": wx,
            "wep": wep,
            "recip": recip_pk[c],
        })
    res = run_bass_kernel_spmd(nc, in_maps, core_ids=list(range(N_CORES)),
                               trace=trace, tmpdir=tmpdir)
    out = np.concatenate([res.results[c]["zout"] for c in range(N_CORES)], axis=0)
    return out, res


def kernel(**inputs) -> np.ndarray:
    out, _ = _run(inputs, trace=False)
    return out
